# revision 31
# baseline (speedup 1.0000x reference)
"""Trainium2 Bass kernel for DiffMultiHeadedAttention (differential attention).

Model (per reference):
    q = x @ Wq.T + bq; k = ef @ Wk.T + bk; v = ef @ Wv.T + bv
    lambda_full = exp(sum(lq1*lk1)) - exp(sum(lq2*lk2)) + 0.8
    att  = softmax(causal_mask(q_hh @ k_hh.T / sqrt(32)))   per 32 half-heads
    out_h = (att[2h] - lambda_full * att[2h+1]) @ v_h       per 16 heads
B=4, T=N=1024, H=16 heads of 64, 2H=32 half-heads of 32.

Sharding over 8 cores: core c = (batch b = c//2, head-group hg = c%2).
Each core owns one batch element and 8 full heads (16 half-heads) and
computes out^T [512, 1024]; the host transposes and reassembles.

Layouts: the host ships activations and weight slices pre-transposed
(contraction-major) so every matmul operand DMA-loads directly; matmul
operands use the f32r (single-pass fp32) PE datapath.

On-core dataflow:
  - qT[o,t] = sum_ic WqT[ic].T @ xT[ic] (+bias), kT likewise from efT,
    v[n,o] = sum_ic efT[ic].T @ WvT[ic] (+bias), augmented with a ones
    column per head.
  - Attention per (head, 512-wide t-chunk): attT[n,t] = kT.T @ qT for the
    un-masked n-tiles (K=32 row-pairs), E = exp(attT/sqrt(32)) with a
    triangular mask multiply on the diagonal 128x128 block, then
    P_psum[65, t] += [v_h | 1].T @ E_s accumulated over n-tiles (row 64 =
    softmax denominators).
  - Inline combine: out = P_pos/s_pos - lambda * P_neg/s_neg via fast DVE
    reciprocal + gpsimd partition-broadcast + one fused DVE multiply; the
    neg half lands via an accumulating SWDGE DMA.
"""

import math

import numpy as np

B, T, N, HIDDEN = 4, 1024, 1024, 1024
H, HEAD, HALF = 16, 64, 32
O = 512            # per-core hidden slice (8 heads * 64)
HPC = 8            # heads per core
LAMBDA_INIT = 0.8
SCALE = 1.0 / math.sqrt(HALF)
P = 128
IC = HIDDEN // P   # 8 contraction chunks
OC = O // P        # 4 output chunks of the projections
NT = N // P        # 8 n-tiles (keys)
NCORES = 8

_STATE = {}


def _build_nc():
    from contextlib import ExitStack

    import concourse.bacc as bacc
    import concourse.mybir as mybir
    import concourse.tile as tile
    from concourse.bass import ts

    f32 = mybir.dt.float32
    f32r = mybir.dt.float32r
    f16 = mybir.dt.float16
    AF = mybir.ActivationFunctionType
    ALU = mybir.AluOpType

    nc = bacc.Bacc("TRN2", target_bir_lowering=False, debug=False)

    xt_d = nc.dram_tensor("xt", [HIDDEN, T], f16, kind="ExternalInput")
    eft_d = nc.dram_tensor("eft", [HIDDEN, N], f16, kind="ExternalInput")
    wqt_d = nc.dram_tensor("wqt", [HIDDEN, O], f16, kind="ExternalInput")
    wkt_d = nc.dram_tensor("wkt", [HIDDEN, O], f16, kind="ExternalInput")
    wvt_d = nc.dram_tensor("wvt", [HIDDEN, O], f16, kind="ExternalInput")
    bq_d = nc.dram_tensor("bq", [1, O], f32, kind="ExternalInput")
    bk_d = nc.dram_tensor("bk", [1, O], f32, kind="ExternalInput")
    bv_d = nc.dram_tensor("bv", [1, O], f32, kind="ExternalInput")
    lq1_d = nc.dram_tensor("lq1", [1, HALF], f32, kind="ExternalInput")
    lq2_d = nc.dram_tensor("lq2", [1, HALF], f32, kind="ExternalInput")
    lk1_d = nc.dram_tensor("lk1", [1, HALF], f32, kind="ExternalInput")
    lk2_d = nc.dram_tensor("lk2", [1, HALF], f32, kind="ExternalInput")
    outT_d = nc.dram_tensor("outT", [O, T], f32, kind="ExternalOutput")

    with tile.TileContext(nc) as tc:
        with ExitStack() as ctx:
            const = ctx.enter_context(tc.tile_pool(name="const", bufs=1))

            # ---- lambda_full (tiny, computed once) ----
            lam_in = const.tile([1, 4, HALF], f32)
            nc.sync.dma_start(lam_in[:, 0, :], lq1_d[:])
            nc.sync.dma_start(lam_in[:, 1, :], lk1_d[:])
            nc.sync.dma_start(lam_in[:, 2, :], lq2_d[:])
            nc.sync.dma_start(lam_in[:, 3, :], lk2_d[:])
            lam_tmp = const.tile([1, 2, HALF], f32)
            nc.vector.tensor_mul(lam_tmp[:, 0, :], lam_in[:, 0, :], lam_in[:, 1, :])
            nc.vector.tensor_mul(lam_tmp[:, 1, :], lam_in[:, 2, :], lam_in[:, 3, :])
            lam_s = const.tile([1, 2], f32)
            nc.vector.tensor_reduce(
                lam_s, lam_tmp, axis=mybir.AxisListType.X, op=ALU.add
            )
            lam_e = const.tile([1, 2], f32)
            nc.scalar.activation(lam_e, lam_s, AF.Exp)
            # lam_neg = -(e1 - e2 + 0.8) = e2 - e1 - 0.8
            lam_neg = const.tile([1, 1], f32)
            nc.vector.tensor_sub(lam_neg, lam_e[:, 1:2], lam_e[:, 0:1])
            nc.vector.tensor_scalar_add(lam_neg, lam_neg, -LAMBDA_INIT)

            # 0/1 upper-triangular mask (keep t_local >= n_local), doubled
            # along a middle dim so one DVE mul masks both half-heads.
            tri2 = const.tile([P, 2, P], f16)
            nc.gpsimd.memset(tri2, 1.0)
            nc.gpsimd.affine_select(
                out=tri2,
                in_=tri2,
                compare_op=ALU.is_ge,
                fill=0.0,
                base=0,
                pattern=[[0, 2], [1, P]],
                channel_multiplier=-1,
            )

            # ---- biases ----
            bq_sb = const.tile([P, OC], f32)
            nc.sync.dma_start(bq_sb, bq_d[0].rearrange("(a p) -> p a", p=P))
            bk_sb = const.tile([P, OC], f32)
            nc.sync.dma_start(bk_sb, bk_d[0].rearrange("(a p) -> p a", p=P))
            bv_1 = const.tile([1, O], f32)
            nc.sync.dma_start(bv_1, bv_d[:])
            bvb = const.tile([P, O], f32)
            nc.gpsimd.partition_broadcast(bvb, bv_1)

            # ---- persistent projection outputs ----
            proj = ctx.enter_context(tc.tile_pool(name="proj", bufs=1))
            qT = proj.tile([P, OC, T], f16)          # [d-part, oc, t]
            kT = proj.tile([P, OC, N], f16)          # [d-part, oc, n]
            vaug = proj.tile([P, NT, HPC, HEAD + 1], f16)  # [n-part, nt, h, d|1]
            ones8 = const.tile([P, HPC], f32)
            nc.vector.memset(ones8, 1.0)
            for nt_ in range(NT):
                # f32r memset fails ISA codegen; route through a DVE copy
                nc.vector.tensor_copy(
                    vaug[:, nt_, :, HEAD : HEAD + 1],
                    ones8[:, :].rearrange("p (a b) -> p a b", b=1),
                )

            # ====== phase 1: load transposed operands + projections ======
            with (
                tc.tile_pool(name="big", bufs=1) as big,
                tc.tile_pool(name="ps_pj", bufs=2, space="PSUM") as ps_pj,
            ):
                efT = big.tile([P, IC, N], f16)
                wvT = big.tile([P, IC, O], f16)
                for ic in range(IC):
                    nc.sync.dma_start(efT[:, ic, :], eft_d[ts(ic, P), :])
                    nc.sync.dma_start(wvT[:, ic, :], wvt_d[ts(ic, P), :])

                # v projection: v[n, o] = sum_ic efT[ic].T @ WvT[ic]  (+bias)
                for nt_ in range(NT):
                    psj = ps_pj.tile([P, 512], f32, tag="pj", name="psv")
                    for ic in range(IC):
                        nc.tensor.matmul(
                            psj,
                            efT[:, ic, ts(nt_, P)],
                            wvT[:, ic, :],
                            start=(ic == 0),
                            stop=(ic == IC - 1),
                        )
                    nc.vector.tensor_add(
                        vaug[:, nt_, :, 0:HEAD],
                        psj[:].rearrange("p (h d) -> p h d", h=HPC),
                        bvb[:].rearrange("p (h d) -> p h d", h=HPC),
                    )

                xT = big.tile([P, IC, T], f16)
                wqT = big.tile([P, IC, O], f16)
                wkT = big.tile([P, IC, O], f16)
                for ic in range(IC):
                    nc.sync.dma_start(xT[:, ic, :], xt_d[ts(ic, P), :])
                    nc.sync.dma_start(wkT[:, ic, :], wkt_d[ts(ic, P), :])
                    nc.sync.dma_start(wqT[:, ic, :], wqt_d[ts(ic, P), :])

                # q/k projections per o-chunk (k first: unblocks attention)
                for oc in range(OC):
                    for wT, b_sb, actT, dstT in (
                        (wkT, bk_sb, efT, kT),
                        (wqT, bq_sb, xT, qT),
                    ):
                        for t2 in range(2):
                            psj = ps_pj.tile([P, 512], f32, tag="pj", name="psj")
                            for ic in range(IC):
                                nc.tensor.matmul(
                                    psj,
                                    wT[:, ic, ts(oc, P)],
                                    actT[:, ic, ts(t2, 512)],
                                    start=(ic == 0),
                                    stop=(ic == IC - 1),
                                )
                            nc.vector.tensor_scalar_add(
                                dstT[:, oc, ts(t2, 512)], psj, b_sb[:, oc : oc + 1]
                            )

            # =============== phase 2: attention ===============
            acc_sb = ctx.enter_context(tc.tile_pool(name="acc_sb", bufs=1))
            # P65[:, h, s, t]: rows 0..63 = (E_s @ v_h).T, row 64 = sum_n E_s
            P65 = acc_sb.tile([65, HPC, 2, T], f32)

            with (
                tc.tile_pool(name="att_sb", bufs=4) as att_sb,
                tc.tile_pool(name="ps_qk", bufs=2, space="PSUM") as ps_qk,
                tc.tile_pool(name="ps_av", bufs=2, space="PSUM") as ps_av,
            ):
                for oc in range(OC):
                    for j in range(2):
                        h = 2 * oc + j
                        for tcv in range(2):
                            avp = ps_av.tile(
                                [65, 2, 512], f32, tag="av", name=f"av{h}_{tcv}"
                            )
                            nis = range(4) if tcv == 0 else range(NT)
                            last = nis[-1]
                            # sweep 1: qk + exp for all n-tiles (uniform PE
                            # geometry back-to-back; E tiles persist in SBUF)
                            Es = {}
                            for nt_ in nis:
                                t0 = nt_ * P
                                cs = max(t0, 512 * tcv)
                                w = 512 * (tcv + 1) - cs
                                att_ps = ps_qk.tile(
                                    [P, 2, 512], f32, tag="qk", name="attps"
                                )
                                E = att_sb.tile(
                                    [P, 2, 512], f16, tag="E", bufs=10, name="E"
                                )
                                Es[nt_] = (E, w)
                                for s in range(2):
                                    base = 64 * j + 32 * s
                                    nc.tensor.matmul(
                                        att_ps[:, s, :w],
                                        kT[base : base + 32, oc, ts(nt_, P)],
                                        qT[base : base + 32, oc, cs : cs + w],
                                        start=True,
                                        stop=True,
                                        tile_position=(96, 0) if base == 96 else None,
                                    )
                                nc.scalar.activation(
                                    E[:, :, :w], att_ps[:, :, :w], AF.Exp, scale=SCALE
                                )
                                if cs == t0:
                                    # diagonal block: keep t_local >= n_local
                                    nc.vector.tensor_mul(
                                        E[:, :, 0:P], E[:, :, 0:P], tri2
                                    )
                            # sweep 2: av accumulation, s-major so PE geometry
                            # and psum bank stay fixed within each run; keep
                            # the run contiguous on PE via a critical section
                            with tc.tile_critical():
                                for s in range(2):
                                    for nt_ in nis:
                                        E, w = Es[nt_]
                                        off = 512 - w
                                        nc.tensor.matmul(
                                            avp[:, s, off : off + w],
                                            vaug[:, nt_, h, :],
                                            E[:, s, :w],
                                            start=(nt_ == 0),
                                            stop=(nt_ == last),
                                        )
                            nc.vector.tensor_copy(
                                P65[:, h, :, ts(tcv, 512)], avp[:, :, :]
                            )
                            # ---- inline combine for this (h, tcv):
                            # out = P_pos/s_pos - lambda * P_neg/s_neg ----
                            Sh = att_sb.tile(
                                [1, 2, 512], f32, tag="Sh", bufs=3, name="Sh"
                            )
                            for s in range(2):
                                nc.sync.dma_start(
                                    Sh[:, s, :], P65[64:65, h, s, ts(tcv, 512)]
                                )
                            Rh = att_sb.tile(
                                [1, 2, 512], f32, tag="Rh", bufs=3, name="Rh"
                            )
                            nc.vector.reciprocal_approx_fast(out=Rh, in_=Sh)
                            nc.vector.tensor_scalar_mul(
                                Rh[:, 1, :], Rh[:, 1, :], lam_neg
                            )
                            Rb = att_sb.tile(
                                [64, 2, 512], f32, tag="Rb", bufs=3, name="Rb"
                            )
                            nc.gpsimd.partition_broadcast(Rb, Rh)
                            m = att_sb.tile(
                                [64, 2, 512], f32, tag="m", bufs=3, name="m"
                            )
                            nc.vector.tensor_mul(
                                m, P65[0:64, h, :, ts(tcv, 512)], Rb
                            )
                            nc.sync.dma_start(
                                outT_d[64 * h : 64 * h + 64, ts(tcv, 512)],
                                m[:, 0, :],
                            )
                            nc.gpsimd.dma_start(
                                outT_d[64 * h : 64 * h + 64, ts(tcv, 512)],
                                m[:, 1, :],
                                accum_op=ALU.add,
                            )

    nc.compile()
    return nc


def _get_state():
    if "nc" not in _STATE:
        from concourse.bass_utils import run_bass_kernel_spmd

        _STATE["nc"] = _build_nc()
        _STATE["run"] = run_bass_kernel_spmd
    return _STATE


def kernel(**inputs):
    st = _get_state()

    def f32c(a):
        return np.ascontiguousarray(np.asarray(a, dtype=np.float32))

    x = np.asarray(inputs["x"], dtype=np.float32)
    ef = np.asarray(inputs["encoder_feature"], dtype=np.float32)
    Wq, bq = np.asarray(inputs["Wq"], np.float32), np.asarray(inputs["bq"], np.float32)
    Wk, bk = np.asarray(inputs["Wk"], np.float32), np.asarray(inputs["bk"], np.float32)
    Wv, bv = np.asarray(inputs["Wv"], np.float32), np.asarray(inputs["bv"], np.float32)
    lq1 = f32c(inputs["lambda_q1"]).reshape(1, HALF)
    lq2 = f32c(inputs["lambda_q2"]).reshape(1, HALF)
    lk1 = f32c(inputs["lambda_k1"]).reshape(1, HALF)
    lk2 = f32c(inputs["lambda_k2"]).reshape(1, HALF)

    in_maps = []
    for c in range(NCORES):
        b, hg = c // 2, c % 2
        sl = slice(hg * O, (hg + 1) * O)
        in_maps.append(
            {
                "xt": np.ascontiguousarray(x[b].T.astype(np.float16)),
                "eft": np.ascontiguousarray(ef[b].T.astype(np.float16)),
                "wqt": np.ascontiguousarray(Wq[sl].T.astype(np.float16)),
                "wkt": np.ascontiguousarray(Wk[sl].T.astype(np.float16)),
                "wvt": np.ascontiguousarray(Wv[sl].T.astype(np.float16)),
                "bq": f32c(bq[sl]).reshape(1, O),
                "bk": f32c(bk[sl]).reshape(1, O),
                "bv": f32c(bv[sl]).reshape(1, O),
                "lq1": lq1,
                "lq2": lq2,
                "lk1": lk1,
                "lk2": lk2,
            }
        )

    res = st["run"](st["nc"], in_maps, core_ids=list(range(NCORES)))
    _STATE["last_results"] = res

    out = np.empty((B, T, HIDDEN), dtype=np.float32)
    for c in range(NCORES):
        b, hg = c // 2, c % 2
        out[b, :, hg * O : (hg + 1) * O] = res.results[c]["outT"].T
    return out


# revision 32
# speedup vs baseline: 1.3493x; 1.3493x over previous
"""Trainium2 Bass kernel for DiffMultiHeadedAttention (differential attention).

Model (per reference):
    q = x @ Wq.T + bq; k = ef @ Wk.T + bk; v = ef @ Wv.T + bv
    lambda_full = exp(sum(lq1*lk1)) - exp(sum(lq2*lk2)) + 0.8
    att  = softmax(causal_mask(q_hh @ k_hh.T / sqrt(32)))   per 32 half-heads
    out_h = (att[2h] - lambda_full * att[2h+1]) @ v_h       per 16 heads
B=4, T=N=1024, H=16 heads of 64, 2H=32 half-heads of 32.

Sharding over 8 cores: core c = (batch b = c//2, head-group hg = c%2).
Each core owns one batch element and 8 full heads (16 half-heads) and
computes out^T [512, 1024]; the host transposes and reassembles.

Layouts: the host ships activations and weight slices pre-transposed
(contraction-major) so every matmul operand DMA-loads directly; matmul
operands use the f32r (single-pass fp32) PE datapath.

On-core dataflow:
  - qT[o,t] = sum_ic WqT[ic].T @ xT[ic] (+bias), kT likewise from efT,
    v[n,o] = sum_ic efT[ic].T @ WvT[ic] (+bias), augmented with a ones
    column per head.
  - Attention per (head, 512-wide t-chunk): attT[n,t] = kT.T @ qT for the
    un-masked n-tiles (K=32 row-pairs), E = exp(attT/sqrt(32)) with a
    triangular mask multiply on the diagonal 128x128 block, then
    P_psum[65, t] += [v_h | 1].T @ E_s accumulated over n-tiles (row 64 =
    softmax denominators).
  - Inline combine: out = P_pos/s_pos - lambda * P_neg/s_neg via fast DVE
    reciprocal + gpsimd partition-broadcast + one fused DVE multiply; the
    neg half lands via an accumulating SWDGE DMA.
"""

import math

import numpy as np

B, T, N, HIDDEN = 4, 1024, 1024, 1024
H, HEAD, HALF = 16, 64, 32
O = 512            # per-core hidden slice (8 heads * 64)
HPC = 8            # heads per core
LAMBDA_INIT = 0.8
SCALE = 1.0 / math.sqrt(HALF)
P = 128
IC = HIDDEN // P   # 8 contraction chunks
OC = O // P        # 4 output chunks of the projections
NT = N // P        # 8 n-tiles (keys)
NCORES = 8

_STATE = {}


def _build_nc():
    from contextlib import ExitStack

    import concourse.bacc as bacc
    import concourse.mybir as mybir
    import concourse.tile as tile
    from concourse.bass import ts

    f32 = mybir.dt.float32
    f32r = mybir.dt.float32r
    f16 = mybir.dt.float16
    AF = mybir.ActivationFunctionType
    ALU = mybir.AluOpType

    nc = bacc.Bacc("TRN2", target_bir_lowering=False, debug=False)

    xt_d = nc.dram_tensor("xt", [HIDDEN, T], f16, kind="ExternalInput")
    eft_d = nc.dram_tensor("eft", [HIDDEN, N], f16, kind="ExternalInput")
    wqt_d = nc.dram_tensor("wqt", [HIDDEN, O], f16, kind="ExternalInput")
    wkt_d = nc.dram_tensor("wkt", [HIDDEN, O], f16, kind="ExternalInput")
    wvt_d = nc.dram_tensor("wvt", [HIDDEN, O], f16, kind="ExternalInput")
    bq_d = nc.dram_tensor("bq", [1, O], f32, kind="ExternalInput")
    bk_d = nc.dram_tensor("bk", [1, O], f32, kind="ExternalInput")
    bv_d = nc.dram_tensor("bv", [1, O], f32, kind="ExternalInput")
    lq1_d = nc.dram_tensor("lq1", [1, HALF], f32, kind="ExternalInput")
    lq2_d = nc.dram_tensor("lq2", [1, HALF], f32, kind="ExternalInput")
    lk1_d = nc.dram_tensor("lk1", [1, HALF], f32, kind="ExternalInput")
    lk2_d = nc.dram_tensor("lk2", [1, HALF], f32, kind="ExternalInput")
    outT_d = nc.dram_tensor("outT", [O, T], f32, kind="ExternalOutput")

    with tile.TileContext(nc) as tc:
        with ExitStack() as ctx:
            const = ctx.enter_context(tc.tile_pool(name="const", bufs=1))

            # ---- lambda_full (tiny, computed once) ----
            lam_in = const.tile([1, 4, HALF], f32)
            nc.sync.dma_start(lam_in[:, 0, :], lq1_d[:])
            nc.sync.dma_start(lam_in[:, 1, :], lk1_d[:])
            nc.sync.dma_start(lam_in[:, 2, :], lq2_d[:])
            nc.sync.dma_start(lam_in[:, 3, :], lk2_d[:])
            lam_tmp = const.tile([1, 2, HALF], f32)
            nc.vector.tensor_mul(lam_tmp[:, 0, :], lam_in[:, 0, :], lam_in[:, 1, :])
            nc.vector.tensor_mul(lam_tmp[:, 1, :], lam_in[:, 2, :], lam_in[:, 3, :])
            lam_s = const.tile([1, 2], f32)
            nc.vector.tensor_reduce(
                lam_s, lam_tmp, axis=mybir.AxisListType.X, op=ALU.add
            )
            lam_e = const.tile([1, 2], f32)
            nc.scalar.activation(lam_e, lam_s, AF.Exp)
            # lam_neg = -(e1 - e2 + 0.8) = e2 - e1 - 0.8
            lam_neg = const.tile([1, 1], f32)
            nc.vector.tensor_sub(lam_neg, lam_e[:, 1:2], lam_e[:, 0:1])
            nc.vector.tensor_scalar_add(lam_neg, lam_neg, -LAMBDA_INIT)

            # 0/1 upper-triangular mask (keep t_local >= n_local), doubled
            # along a middle dim so one DVE mul masks both half-heads.
            tri2 = const.tile([P, 2, P], f16)
            nc.gpsimd.memset(tri2, 1.0)
            nc.gpsimd.affine_select(
                out=tri2,
                in_=tri2,
                compare_op=ALU.is_ge,
                fill=0.0,
                base=0,
                pattern=[[0, 2], [1, P]],
                channel_multiplier=-1,
            )

            # ---- biases ----
            bq_sb = const.tile([P, OC], f32)
            nc.sync.dma_start(bq_sb, bq_d[0].rearrange("(a p) -> p a", p=P))
            bk_sb = const.tile([P, OC], f32)
            nc.sync.dma_start(bk_sb, bk_d[0].rearrange("(a p) -> p a", p=P))
            bv_1 = const.tile([1, O], f32)
            nc.sync.dma_start(bv_1, bv_d[:])
            bvb = const.tile([P, O], f32)
            nc.gpsimd.partition_broadcast(bvb, bv_1)

            # ---- persistent projection outputs ----
            proj = ctx.enter_context(tc.tile_pool(name="proj", bufs=1))
            qT = proj.tile([P, OC, T], f16)          # [d-part, oc, t]
            kT = proj.tile([P, OC, N], f16)          # [d-part, oc, n]
            vaug = proj.tile([P, NT, HPC, HEAD + 1], f16)  # [n-part, nt, h, d|1]
            ones8 = const.tile([P, HPC], f32)
            nc.vector.memset(ones8, 1.0)
            for nt_ in range(NT):
                # f32r memset fails ISA codegen; route through a DVE copy
                nc.vector.tensor_copy(
                    vaug[:, nt_, :, HEAD : HEAD + 1],
                    ones8[:, :].rearrange("p (a b) -> p a b", b=1),
                )

            # ====== phase 1: load transposed operands + projections ======
            with (
                tc.tile_pool(name="big", bufs=1) as big,
                tc.tile_pool(name="ps_pj", bufs=2, space="PSUM") as ps_pj,
            ):
                efT = big.tile([P, IC, N], f16)
                wvT = big.tile([P, IC, O], f16)
                for ic in range(IC):
                    nc.sync.dma_start(efT[:, ic, :], eft_d[ts(ic, P), :])
                    nc.sync.dma_start(wvT[:, ic, :], wvt_d[ts(ic, P), :])

                # v projection: v[n, o] = sum_ic efT[ic].T @ WvT[ic]  (+bias)
                for nt_ in range(NT):
                    psj = ps_pj.tile([P, 512], f32, tag="pj", name="psv")
                    for ic in range(IC):
                        nc.tensor.matmul(
                            psj,
                            efT[:, ic, ts(nt_, P)],
                            wvT[:, ic, :],
                            start=(ic == 0),
                            stop=(ic == IC - 1),
                        )
                    nc.vector.tensor_add(
                        vaug[:, nt_, :, 0:HEAD],
                        psj[:].rearrange("p (h d) -> p h d", h=HPC),
                        bvb[:].rearrange("p (h d) -> p h d", h=HPC),
                    )

                xT = big.tile([P, IC, T], f16)
                wqT = big.tile([P, IC, O], f16)
                wkT = big.tile([P, IC, O], f16)
                for ic in range(IC):
                    nc.sync.dma_start(xT[:, ic, :], xt_d[ts(ic, P), :])
                    nc.sync.dma_start(wkT[:, ic, :], wkt_d[ts(ic, P), :])
                    nc.sync.dma_start(wqT[:, ic, :], wqt_d[ts(ic, P), :])

                # q/k projections per o-chunk (k first: unblocks attention)
                for oc in range(OC):
                    for wT, b_sb, actT, dstT in (
                        (wkT, bk_sb, efT, kT),
                        (wqT, bq_sb, xT, qT),
                    ):
                        for t2 in range(2):
                            psj = ps_pj.tile([P, 512], f32, tag="pj", name="psj")
                            for ic in range(IC):
                                nc.tensor.matmul(
                                    psj,
                                    wT[:, ic, ts(oc, P)],
                                    actT[:, ic, ts(t2, 512)],
                                    start=(ic == 0),
                                    stop=(ic == IC - 1),
                                )
                            nc.vector.tensor_scalar_add(
                                dstT[:, oc, ts(t2, 512)], psj, b_sb[:, oc : oc + 1]
                            )

            # =============== phase 2: attention ===============
            acc_sb = ctx.enter_context(tc.tile_pool(name="acc_sb", bufs=1))
            # P65[:, h, s, t]: rows 0..63 = (E_s @ v_h).T, row 64 = sum_n E_s
            P65 = acc_sb.tile([65, HPC, 2, T], f32)

            with (
                tc.tile_pool(name="att_sb", bufs=4) as att_sb,
                tc.tile_pool(name="ps_qk", bufs=2, space="PSUM") as ps_qk,
                tc.tile_pool(name="ps_av", bufs=2, space="PSUM") as ps_av,
            ):
                for oc in range(OC):
                    for j in range(2):
                        h = 2 * oc + j
                        for tcv in range(2):
                            avp = ps_av.tile(
                                [65, 2, 512], f32, tag="av", name=f"av{h}_{tcv}"
                            )
                            nis = range(4) if tcv == 0 else range(NT)
                            last = nis[-1]
                            # sweep 1: qk + exp for all n-tiles (uniform PE
                            # geometry back-to-back; E tiles persist in SBUF)
                            Es = {}
                            for nt_ in nis:
                                t0 = nt_ * P
                                cs = max(t0, 512 * tcv)
                                w = 512 * (tcv + 1) - cs
                                att_ps = ps_qk.tile(
                                    [P, 2, 512], f32, tag="qk", name="attps"
                                )
                                E = att_sb.tile(
                                    [P, 2, 512], f16, tag="E", bufs=10, name="E"
                                )
                                Es[nt_] = (E, w)
                                for s in range(2):
                                    base = 64 * j + 32 * s
                                    nc.tensor.matmul(
                                        att_ps[:, s, :w],
                                        kT[base : base + 32, oc, ts(nt_, P)],
                                        qT[base : base + 32, oc, cs : cs + w],
                                        start=True,
                                        stop=True,
                                        tile_position=(96, 0) if base == 96 else None,
                                    )
                                nc.scalar.activation(
                                    E[:, :, :w], att_ps[:, :, :w], AF.Exp, scale=SCALE
                                )
                                if cs == t0:
                                    # diagonal block: keep t_local >= n_local
                                    nc.vector.tensor_mul(
                                        E[:, :, 0:P], E[:, :, 0:P], tri2
                                    )
                            # sweep 2: av accumulation, s-major so PE geometry
                            # and psum bank stay fixed within each run
                            for s in range(2):
                                for nt_ in nis:
                                    E, w = Es[nt_]
                                    off = 512 - w
                                    nc.tensor.matmul(
                                        avp[:, s, off : off + w],
                                        vaug[:, nt_, h, :],
                                        E[:, s, :w],
                                        start=(nt_ == 0),
                                        stop=(nt_ == last),
                                    )
                            nc.vector.tensor_copy(
                                P65[:, h, :, ts(tcv, 512)], avp[:, :, :]
                            )
                            # ---- inline combine for this (h, tcv):
                            # out = P_pos/s_pos - lambda * P_neg/s_neg ----
                            Sh = att_sb.tile(
                                [1, 2, 512], f32, tag="Sh", bufs=3, name="Sh"
                            )
                            for s in range(2):
                                nc.sync.dma_start(
                                    Sh[:, s, :], P65[64:65, h, s, ts(tcv, 512)]
                                )
                            Rh = att_sb.tile(
                                [1, 2, 512], f32, tag="Rh", bufs=3, name="Rh"
                            )
                            nc.vector.reciprocal_approx_fast(out=Rh, in_=Sh)
                            nc.vector.tensor_scalar_mul(
                                Rh[:, 1, :], Rh[:, 1, :], lam_neg
                            )
                            Rb = att_sb.tile(
                                [64, 2, 512], f32, tag="Rb", bufs=3, name="Rb"
                            )
                            nc.gpsimd.partition_broadcast(Rb, Rh)
                            m = att_sb.tile(
                                [64, 2, 512], f32, tag="m", bufs=3, name="m"
                            )
                            nc.vector.tensor_mul(
                                m, P65[0:64, h, :, ts(tcv, 512)], Rb
                            )
                            nc.sync.dma_start(
                                outT_d[64 * h : 64 * h + 64, ts(tcv, 512)],
                                m[:, 0, :],
                            )
                            nc.gpsimd.dma_start(
                                outT_d[64 * h : 64 * h + 64, ts(tcv, 512)],
                                m[:, 1, :],
                                accum_op=ALU.add,
                            )

    nc.compile()
    return nc


def _get_state():
    if "nc" not in _STATE:
        from concourse.bass_utils import run_bass_kernel_spmd

        _STATE["nc"] = _build_nc()
        _STATE["run"] = run_bass_kernel_spmd
    return _STATE


def kernel(**inputs):
    st = _get_state()

    def f32c(a):
        return np.ascontiguousarray(np.asarray(a, dtype=np.float32))

    x = np.asarray(inputs["x"], dtype=np.float32)
    ef = np.asarray(inputs["encoder_feature"], dtype=np.float32)
    Wq, bq = np.asarray(inputs["Wq"], np.float32), np.asarray(inputs["bq"], np.float32)
    Wk, bk = np.asarray(inputs["Wk"], np.float32), np.asarray(inputs["bk"], np.float32)
    Wv, bv = np.asarray(inputs["Wv"], np.float32), np.asarray(inputs["bv"], np.float32)
    lq1 = f32c(inputs["lambda_q1"]).reshape(1, HALF)
    lq2 = f32c(inputs["lambda_q2"]).reshape(1, HALF)
    lk1 = f32c(inputs["lambda_k1"]).reshape(1, HALF)
    lk2 = f32c(inputs["lambda_k2"]).reshape(1, HALF)

    in_maps = []
    for c in range(NCORES):
        b, hg = c // 2, c % 2
        sl = slice(hg * O, (hg + 1) * O)
        in_maps.append(
            {
                "xt": np.ascontiguousarray(x[b].T.astype(np.float16)),
                "eft": np.ascontiguousarray(ef[b].T.astype(np.float16)),
                "wqt": np.ascontiguousarray(Wq[sl].T.astype(np.float16)),
                "wkt": np.ascontiguousarray(Wk[sl].T.astype(np.float16)),
                "wvt": np.ascontiguousarray(Wv[sl].T.astype(np.float16)),
                "bq": f32c(bq[sl]).reshape(1, O),
                "bk": f32c(bk[sl]).reshape(1, O),
                "bv": f32c(bv[sl]).reshape(1, O),
                "lq1": lq1,
                "lq2": lq2,
                "lk1": lk1,
                "lk2": lk2,
            }
        )

    res = st["run"](st["nc"], in_maps, core_ids=list(range(NCORES)))
    _STATE["last_results"] = res

    out = np.empty((B, T, HIDDEN), dtype=np.float32)
    for c in range(NCORES):
        b, hg = c // 2, c % 2
        out[b, :, hg * O : (hg + 1) * O] = res.results[c]["outT"].T
    return out


# revision 35
# speedup vs baseline: 1.3545x; 1.0038x over previous
"""Trainium2 Bass kernel for DiffMultiHeadedAttention (differential attention).

Model (per reference):
    q = x @ Wq.T + bq; k = ef @ Wk.T + bk; v = ef @ Wv.T + bv
    lambda_full = exp(sum(lq1*lk1)) - exp(sum(lq2*lk2)) + 0.8
    att  = softmax(causal_mask(q_hh @ k_hh.T / sqrt(32)))   per 32 half-heads
    out_h = (att[2h] - lambda_full * att[2h+1]) @ v_h       per 16 heads
B=4, T=N=1024, H=16 heads of 64, 2H=32 half-heads of 32.

Sharding over 8 cores: core c = (batch b = c//2, head-group hg = c%2).
Each core owns one batch element and 8 full heads (16 half-heads) and
computes out^T [512, 1024]; the host transposes and reassembles.

Layouts: the host ships activations and weight slices pre-transposed
(contraction-major) so every matmul operand DMA-loads directly; matmul
operands use the f32r (single-pass fp32) PE datapath.

On-core dataflow:
  - qT[o,t] = sum_ic WqT[ic].T @ xT[ic] (+bias), kT likewise from efT,
    v[n,o] = sum_ic efT[ic].T @ WvT[ic] (+bias), augmented with a ones
    column per head.
  - Attention per (head, 512-wide t-chunk): attT[n,t] = kT.T @ qT for the
    un-masked n-tiles (K=32 row-pairs), E = exp(attT/sqrt(32)) with a
    triangular mask multiply on the diagonal 128x128 block, then
    P_psum[65, t] += [v_h | 1].T @ E_s accumulated over n-tiles (row 64 =
    softmax denominators).
  - Inline combine: out = P_pos/s_pos - lambda * P_neg/s_neg via fast DVE
    reciprocal + gpsimd partition-broadcast + one fused DVE multiply; the
    neg half lands via an accumulating SWDGE DMA.
"""

import math

import numpy as np

B, T, N, HIDDEN = 4, 1024, 1024, 1024
H, HEAD, HALF = 16, 64, 32
O = 512            # per-core hidden slice (8 heads * 64)
HPC = 8            # heads per core
LAMBDA_INIT = 0.8
SCALE = 1.0 / math.sqrt(HALF)
P = 128
IC = HIDDEN // P   # 8 contraction chunks
OC = O // P        # 4 output chunks of the projections
NT = N // P        # 8 n-tiles (keys)
NCORES = 8

_STATE = {}


def _build_nc():
    from contextlib import ExitStack

    import concourse.bacc as bacc
    import concourse.mybir as mybir
    import concourse.tile as tile
    from concourse.bass import ts

    f32 = mybir.dt.float32
    f32r = mybir.dt.float32r
    f16 = mybir.dt.float16
    AF = mybir.ActivationFunctionType
    ALU = mybir.AluOpType

    nc = bacc.Bacc("TRN2", target_bir_lowering=False, debug=False)

    xt_d = nc.dram_tensor("xt", [HIDDEN, T], f16, kind="ExternalInput")
    eft_d = nc.dram_tensor("eft", [HIDDEN, N], f16, kind="ExternalInput")
    wqt_d = nc.dram_tensor("wqt", [HIDDEN, O], f16, kind="ExternalInput")
    wkt_d = nc.dram_tensor("wkt", [HIDDEN, O], f16, kind="ExternalInput")
    wvt_d = nc.dram_tensor("wvt", [HIDDEN, O], f16, kind="ExternalInput")
    bq_d = nc.dram_tensor("bq", [1, O], f32, kind="ExternalInput")
    bk_d = nc.dram_tensor("bk", [1, O], f32, kind="ExternalInput")
    bv_d = nc.dram_tensor("bv", [1, O], f32, kind="ExternalInput")
    lq1_d = nc.dram_tensor("lq1", [1, HALF], f32, kind="ExternalInput")
    lq2_d = nc.dram_tensor("lq2", [1, HALF], f32, kind="ExternalInput")
    lk1_d = nc.dram_tensor("lk1", [1, HALF], f32, kind="ExternalInput")
    lk2_d = nc.dram_tensor("lk2", [1, HALF], f32, kind="ExternalInput")
    outT_d = nc.dram_tensor("outT", [O, T], f32, kind="ExternalOutput")

    with tile.TileContext(nc) as tc:
        with ExitStack() as ctx:
            const = ctx.enter_context(tc.tile_pool(name="const", bufs=1))

            # ---- lambda_full (tiny, computed once) ----
            lam_in = const.tile([1, 4, HALF], f32)
            nc.sync.dma_start(lam_in[:, 0, :], lq1_d[:])
            nc.sync.dma_start(lam_in[:, 1, :], lk1_d[:])
            nc.sync.dma_start(lam_in[:, 2, :], lq2_d[:])
            nc.sync.dma_start(lam_in[:, 3, :], lk2_d[:])
            lam_tmp = const.tile([1, 2, HALF], f32)
            nc.vector.tensor_mul(lam_tmp[:, 0, :], lam_in[:, 0, :], lam_in[:, 1, :])
            nc.vector.tensor_mul(lam_tmp[:, 1, :], lam_in[:, 2, :], lam_in[:, 3, :])
            lam_s = const.tile([1, 2], f32)
            nc.vector.tensor_reduce(
                lam_s, lam_tmp, axis=mybir.AxisListType.X, op=ALU.add
            )
            lam_e = const.tile([1, 2], f32)
            nc.scalar.activation(lam_e, lam_s, AF.Exp)
            # lam_neg = -(e1 - e2 + 0.8) = e2 - e1 - 0.8
            lam_neg = const.tile([1, 1], f32)
            nc.vector.tensor_sub(lam_neg, lam_e[:, 1:2], lam_e[:, 0:1])
            nc.vector.tensor_scalar_add(lam_neg, lam_neg, -LAMBDA_INIT)

            # 0/1 upper-triangular mask (keep t_local >= n_local), doubled
            # along a middle dim so one DVE mul masks both half-heads.
            tri2 = const.tile([P, 2, P], f16)
            neg3 = const.tile([P, 1], f32)
            nc.vector.memset(neg3, -3.0)
            nc.gpsimd.memset(tri2, 1.0)
            nc.gpsimd.affine_select(
                out=tri2,
                in_=tri2,
                compare_op=ALU.is_ge,
                fill=0.0,
                base=0,
                pattern=[[0, 2], [1, P]],
                channel_multiplier=-1,
            )

            # ---- biases ----
            bq_sb = const.tile([P, OC], f32)
            nc.sync.dma_start(bq_sb, bq_d[0].rearrange("(a p) -> p a", p=P))
            bk_sb = const.tile([P, OC], f32)
            nc.sync.dma_start(bk_sb, bk_d[0].rearrange("(a p) -> p a", p=P))
            bv_1 = const.tile([1, O], f32)
            nc.sync.dma_start(bv_1, bv_d[:])
            bvb = const.tile([P, O], f32)
            nc.gpsimd.partition_broadcast(bvb, bv_1)

            # ---- persistent projection outputs ----
            proj = ctx.enter_context(tc.tile_pool(name="proj", bufs=1))
            qT = proj.tile([P, OC, T], f16)          # [d-part, oc, t]
            kT = proj.tile([P, OC, N], f16)          # [d-part, oc, n]
            vaug = proj.tile([P, NT, HPC, HEAD + 1], f16)  # [n-part, nt, h, d|1]
            ones8 = const.tile([P, HPC], f32)
            nc.vector.memset(ones8, 1.0)
            for nt_ in range(NT):
                # f32r memset fails ISA codegen; route through a DVE copy
                nc.vector.tensor_copy(
                    vaug[:, nt_, :, HEAD : HEAD + 1],
                    ones8[:, :].rearrange("p (a b) -> p a b", b=1),
                )

            # ====== phase 1: load transposed operands + projections ======
            with (
                tc.tile_pool(name="big", bufs=1) as big,
                tc.tile_pool(name="ps_pj", bufs=2, space="PSUM") as ps_pj,
            ):
                efT = big.tile([P, IC, N], f16)
                wvT = big.tile([P, IC, O], f16)
                for ic in range(IC):
                    nc.sync.dma_start(efT[:, ic, :], eft_d[ts(ic, P), :])
                    nc.sync.dma_start(wvT[:, ic, :], wvt_d[ts(ic, P), :])

                # v projection: v[n, o] = sum_ic efT[ic].T @ WvT[ic]  (+bias)
                for nt_ in range(NT):
                    psj = ps_pj.tile([P, 512], f32, tag="pj", name="psv")
                    for ic in range(IC):
                        nc.tensor.matmul(
                            psj,
                            efT[:, ic, ts(nt_, P)],
                            wvT[:, ic, :],
                            start=(ic == 0),
                            stop=(ic == IC - 1),
                        )
                    nc.vector.tensor_add(
                        vaug[:, nt_, :, 0:HEAD],
                        psj[:].rearrange("p (h d) -> p h d", h=HPC),
                        bvb[:].rearrange("p (h d) -> p h d", h=HPC),
                    )

                xT = big.tile([P, IC, T], f16)
                wqT = big.tile([P, IC, O], f16)
                wkT = big.tile([P, IC, O], f16)
                for ic in range(IC):
                    nc.sync.dma_start(xT[:, ic, :], xt_d[ts(ic, P), :])
                    nc.sync.dma_start(wkT[:, ic, :], wkt_d[ts(ic, P), :])
                    nc.sync.dma_start(wqT[:, ic, :], wqt_d[ts(ic, P), :])

                # q/k projections per o-chunk (k first: unblocks attention)
                for oc in range(OC):
                    for wT, b_sb, actT, dstT in (
                        (wkT, bk_sb, efT, kT),
                        (wqT, bq_sb, xT, qT),
                    ):
                        for t2 in range(2):
                            psj = ps_pj.tile([P, 512], f32, tag="pj", name="psj")
                            for ic in range(IC):
                                nc.tensor.matmul(
                                    psj,
                                    wT[:, ic, ts(oc, P)],
                                    actT[:, ic, ts(t2, 512)],
                                    start=(ic == 0),
                                    stop=(ic == IC - 1),
                                )
                            nc.vector.tensor_scalar_add(
                                dstT[:, oc, ts(t2, 512)], psj, b_sb[:, oc : oc + 1]
                            )

            # =============== phase 2: attention ===============
            acc_sb = ctx.enter_context(tc.tile_pool(name="acc_sb", bufs=1))
            # P65[:, h, s, t]: rows 0..63 = (E_s @ v_h).T, row 64 = sum_n E_s
            P65 = acc_sb.tile([65, HPC, 2, T], f32)

            with (
                tc.tile_pool(name="att_sb", bufs=4) as att_sb,
                tc.tile_pool(name="ps_qk", bufs=2, space="PSUM") as ps_qk,
                tc.tile_pool(name="ps_av", bufs=2, space="PSUM") as ps_av,
            ):
                for oc in range(OC):
                    for j in range(2):
                        h = 2 * oc + j
                        for tcv in (1, 0):
                            avp = ps_av.tile(
                                [65, 2, 512], f32, tag="av", name=f"av{h}_{tcv}"
                            )
                            nis = range(4) if tcv == 0 else range(NT)
                            last = nis[-1]
                            # sweep 1: qk + exp for all n-tiles (uniform PE
                            # geometry back-to-back; E tiles persist in SBUF)
                            Es = {}
                            for nt_ in nis:
                                t0 = nt_ * P
                                cs = max(t0, 512 * tcv)
                                w = 512 * (tcv + 1) - cs
                                att_ps = ps_qk.tile(
                                    [P, 2, 512], f32, tag="qk", name="attps"
                                )
                                E = att_sb.tile(
                                    [P, 2, 512], f16, tag="E", bufs=10, name="E"
                                )
                                Es[nt_] = (E, w)
                                for s in range(2):
                                    base = 64 * j + 32 * s
                                    nc.tensor.matmul(
                                        att_ps[:, s, :w],
                                        kT[base : base + 32, oc, ts(nt_, P)],
                                        qT[base : base + 32, oc, cs : cs + w],
                                        start=True,
                                        stop=True,
                                        tile_position=(96, 0) if base == 96 else None,
                                    )
                                # bias shifts all exps by e^-3 (cancels in
                                # P/s) keeping E inside fp16 range
                                nc.scalar.activation(
                                    E[:, :, :w],
                                    att_ps[:, :, :w],
                                    AF.Exp,
                                    bias=neg3[:, 0:1],
                                    scale=SCALE,
                                )
                                if cs == t0:
                                    # diagonal block: keep t_local >= n_local
                                    nc.vector.tensor_mul(
                                        E[:, :, 0:P], E[:, :, 0:P], tri2
                                    )
                            # sweep 2: av accumulation, s-major so PE geometry
                            # and psum bank stay fixed within each run
                            for s in range(2):
                                for nt_ in nis:
                                    E, w = Es[nt_]
                                    off = 512 - w
                                    nc.tensor.matmul(
                                        avp[:, s, off : off + w],
                                        vaug[:, nt_, h, :],
                                        E[:, s, :w],
                                        start=(nt_ == 0),
                                        stop=(nt_ == last),
                                    )
                            nc.vector.tensor_copy(
                                P65[:, h, :, ts(tcv, 512)], avp[:, :, :]
                            )
                            # ---- inline combine for this (h, tcv):
                            # out = P_pos/s_pos - lambda * P_neg/s_neg ----
                            Sh = att_sb.tile(
                                [1, 2, 512], f32, tag="Sh", bufs=3, name="Sh"
                            )
                            for s in range(2):
                                nc.sync.dma_start(
                                    Sh[:, s, :], P65[64:65, h, s, ts(tcv, 512)]
                                )
                            Rh = att_sb.tile(
                                [1, 2, 512], f32, tag="Rh", bufs=3, name="Rh"
                            )
                            nc.vector.reciprocal_approx_fast(out=Rh, in_=Sh)
                            nc.vector.tensor_scalar_mul(
                                Rh[:, 1, :], Rh[:, 1, :], lam_neg
                            )
                            Rb = att_sb.tile(
                                [64, 2, 512], f32, tag="Rb", bufs=3, name="Rb"
                            )
                            nc.gpsimd.partition_broadcast(Rb, Rh)
                            m = att_sb.tile(
                                [64, 2, 512], f32, tag="m", bufs=3, name="m"
                            )
                            nc.vector.tensor_mul(
                                m, P65[0:64, h, :, ts(tcv, 512)], Rb
                            )
                            nc.sync.dma_start(
                                outT_d[64 * h : 64 * h + 64, ts(tcv, 512)],
                                m[:, 0, :],
                            )
                            nc.gpsimd.dma_start(
                                outT_d[64 * h : 64 * h + 64, ts(tcv, 512)],
                                m[:, 1, :],
                                accum_op=ALU.add,
                            )

    nc.compile()
    return nc


def _get_state():
    if "nc" not in _STATE:
        from concourse.bass_utils import run_bass_kernel_spmd

        _STATE["nc"] = _build_nc()
        _STATE["run"] = run_bass_kernel_spmd
    return _STATE


def kernel(**inputs):
    st = _get_state()

    def f32c(a):
        return np.ascontiguousarray(np.asarray(a, dtype=np.float32))

    x = np.asarray(inputs["x"], dtype=np.float32)
    ef = np.asarray(inputs["encoder_feature"], dtype=np.float32)
    Wq, bq = np.asarray(inputs["Wq"], np.float32), np.asarray(inputs["bq"], np.float32)
    Wk, bk = np.asarray(inputs["Wk"], np.float32), np.asarray(inputs["bk"], np.float32)
    Wv, bv = np.asarray(inputs["Wv"], np.float32), np.asarray(inputs["bv"], np.float32)
    lq1 = f32c(inputs["lambda_q1"]).reshape(1, HALF)
    lq2 = f32c(inputs["lambda_q2"]).reshape(1, HALF)
    lk1 = f32c(inputs["lambda_k1"]).reshape(1, HALF)
    lk2 = f32c(inputs["lambda_k2"]).reshape(1, HALF)

    in_maps = []
    for c in range(NCORES):
        b, hg = c // 2, c % 2
        sl = slice(hg * O, (hg + 1) * O)
        in_maps.append(
            {
                "xt": np.ascontiguousarray(x[b].T.astype(np.float16)),
                "eft": np.ascontiguousarray(ef[b].T.astype(np.float16)),
                "wqt": np.ascontiguousarray(Wq[sl].T.astype(np.float16)),
                "wkt": np.ascontiguousarray(Wk[sl].T.astype(np.float16)),
                "wvt": np.ascontiguousarray(Wv[sl].T.astype(np.float16)),
                "bq": f32c(bq[sl]).reshape(1, O),
                "bk": f32c(bk[sl]).reshape(1, O),
                "bv": f32c(bv[sl]).reshape(1, O),
                "lq1": lq1,
                "lq2": lq2,
                "lk1": lk1,
                "lk2": lk2,
            }
        )

    res = st["run"](st["nc"], in_maps, core_ids=list(range(NCORES)))
    _STATE["last_results"] = res

    out = np.empty((B, T, HIDDEN), dtype=np.float32)
    for c in range(NCORES):
        b, hg = c // 2, c % 2
        out[b, :, hg * O : (hg + 1) * O] = res.results[c]["outT"].T
    return out
